# revision 20
# baseline (speedup 1.0000x reference)
"""DiT forward kernel for 8 Trainium2 NeuronCores.

Sharding: data-parallel over batch B=8 (one batch element per core).
Per-core layout: activations live transposed ("T-domain") in SBUF as
[D on partitions, tokens on free]. All projection matmuls then use the
weight matrix as stored ([Din, Dout]) for lhsT and the activation as the
moving operand (or vice versa), so no activation transposes are needed.

Attention: scores are computed in [j, i] layout (lhsT = kT slice,
rhs = qT), exp on ScalarE with no max-subtraction (|s| <= ~0.16 by
construction), the softmax denominator comes free from a ones-column
appended to V, and normalization is applied after accumulating
o = exp(s) @ V via a PE row-broadcast of 1/Z.

All matmuls are bf16 (PSUM accumulates fp32). The two precision-critical
matmuls (patch embed, final projection) use a bf16 hi+lo split (~fp32
quality). The residual stream stays fp32 in SBUF.
"""

import math
import os
import sys
import types

sys.path.insert(0, "/opt/trn_rl_repo")

import numpy as np
import ml_dtypes

import concourse.bass as bass
import concourse.tile as tile
from concourse import bacc, mybir
from concourse.bass_utils import run_bass_kernel_spmd

F32 = mybir.dt.float32
BF16 = mybir.dt.bfloat16
AF = mybir.ActivationFunctionType
OP = mybir.AluOpType

B = 8
CIN = 4
IMG = 64
P = 2
D = 384
NH = 6
L = 12
MLP = 4 * D
NCLS = 1000
FREQ = 256
COUT = 4
HP = IMG // P
N = HP * HP  # 1024 tokens
HD = D // NH  # 64
KT = D // 128  # 3 k-tiles of the model dim
MT1 = MLP // 128  # 12 m-tiles of the MLP dim
NHALF = N // 512  # 2 column halves of 512 tokens


def _register_profile_hook():
    """The stub antenv lacks axon_hooks; register the NTFF hook ourselves."""
    if "antenv.axon_hooks" in sys.modules:
        return
    try:
        import antenv
        from trn_agent_boot.trn_boot import _ntff_profile_via_ctypes

        mod = types.ModuleType("antenv.axon_hooks")
        mod._hook = _ntff_profile_via_ctypes("/opt/axon/libaxon_pjrt.so")
        mod.get_axon_ntff_profile_hook = lambda: mod._hook
        mod.set_axon_ntff_profile_hook = lambda h: setattr(mod, "_hook", h)
        sys.modules["antenv.axon_hooks"] = mod
        antenv.axon_hooks = mod
    except Exception:
        pass


def build_module(n_layers=L):
    """Emit the full per-core BIR module. Returns the Bacc."""
    nc = bacc.Bacc(None, target_bir_lowering=False)

    def din(name, shape, dtype=F32):
        return nc.declare_dram_parameter(name, list(shape), dtype, isOutput=False)

    # ---- per-core inputs ----
    xpT_hi = din("xpT_hi", [16, N], BF16)
    xpT_lo = din("xpT_lo", [16, N], BF16)
    t_in = din("t_in", [1, 1])
    dt_in = din("dt_in", [1, 1])
    cbias_in = din("cbias", [1, D])  # t1_b2 + t2_b2 + label_emb[y]

    # ---- shared weights (host pre-shuffled) ----
    posT_in = din("posT", [128, KT, N])
    pw_hi = din("pw_hi", [16, D], BF16)
    pw_lo = din("pw_lo", [16, D], BF16)
    patch_b_in = din("patch_b", [128, KT])
    t1w1_in = din("t1w1", [128, 2, D], BF16)
    t1w2_in = din("t1w2", [128, KT, D], BF16)
    t2w1_in = din("t2w1", [128, 2, D], BF16)
    t2w2_in = din("t2w2", [128, KT, D], BF16)
    t1b1_in = din("t1b1", [1, D])
    t2b1_in = din("t2b1", [1, D])
    freqs_in = din("freqs", [128, 1])
    adaln_w_in = din("adaln_w", [L, 128, KT, 6 * D], BF16)
    adaln_b_in = din("adaln_b", [L, 128, 18])
    qw_in = din("qw", [L, 128, KT, D], BF16)
    kw_in = din("kw", [L, 128, KT, D], BF16)
    vw_in = din("vw", [L, 128, KT, D], BF16)
    ow_in = din("ow", [L, 128, KT, D], BF16)
    qb_in = din("qb", [L, 128, KT])
    kb_in = din("kb", [L, 128, KT])
    vb_row_in = din("vb_row", [L, 1, D])
    ob_in = din("ob", [L, 128, KT])
    mw1_in = din("mw1", [L, 128, KT, MLP], BF16)
    mw2_in = din("mw2", [L, 128, MT1, D], BF16)
    mb1_in = din("mb1", [L, 128, MT1])
    mb2_in = din("mb2", [L, 128, KT])
    finmw_in = din("finmw", [128, KT, 2 * D], BF16)
    finmb_in = din("finmb", [128, 6])
    fpw_hi = din("fpw_hi", [128, KT, 16], BF16)
    fpw_lo = din("fpw_lo", [128, KT, 16], BF16)
    fpb_in = din("fpb", [16, 1])

    outT = nc.declare_dram_parameter("outT", [16, N], F32, isOutput=True)
    DBG = os.environ.get("DIT_DBG", "0") == "1"
    if DBG:
        dbg_h0 = nc.declare_dram_parameter("dbg_h0", [128, KT, N], F32, isOutput=True)
        dbg_fincol = nc.declare_dram_parameter("dbg_fincol", [128, 6], F32, isOutput=True)
        dbg_xmF = nc.declare_dram_parameter("dbg_xmF", [128, KT, N], F32, isOutput=True)
        dbg_ca = nc.declare_dram_parameter("dbg_ca", [128, KT], F32, isOutput=True)

    with tile.TileContext(nc) as tc:
        _emit(nc, tc, locals(), n_layers)
    nc.compile()
    return nc


def _emit(nc, tc, H, n_layers):
    import contextlib

    ctx = contextlib.ExitStack()
    with ctx:
        # ---------------- pools ----------------
        const = ctx.enter_context(tc.tile_pool(name="const", bufs=1))
        persist = ctx.enter_context(tc.tile_pool(name="persist", bufs=1))
        wpool = ctx.enter_context(tc.tile_pool(name="wpool", bufs=2))
        act16 = ctx.enter_context(tc.tile_pool(name="act16", bufs=1))
        stat = ctx.enter_context(tc.tile_pool(name="stat", bufs=2))
        rowp = ctx.enter_context(tc.tile_pool(name="rowp", bufs=2))
        expp = ctx.enter_context(tc.tile_pool(name="expp", bufs=3))
        resid = ctx.enter_context(tc.tile_pool(name="resid", bufs=2))
        ps = ctx.enter_context(tc.tile_pool(name="ps", bufs=1, space="PSUM"))
        dramp = ctx.enter_context(tc.tile_pool(name="dramp", bufs=2, space="DRAM"))

        def row_to_col(row, col, nelem, dtype, nm):
            """[1, nelem] sbuf row -> [128, nelem//128] sbuf column layout."""
            scr = dramp.tile([nelem], dtype, tag="scr", name=f"scr_{nm}")
            nc.sync.dma_start(out=scr, in_=row[:1, :nelem])
            nc.sync.dma_start(
                out=col,
                in_=bass.AP(
                    tensor=scr.tensor, offset=scr.offset,
                    ap=[[1, 128], [128, nelem // 128]],
                ),
            )

        def psum_s(shape=None):
            return ps.tile(shape or [128, N], F32, tag="s", bufs=2, name="ps_s")

        def psum_o(shape=None):
            return ps.tile(shape or [65, N], F32, tag="o", bufs=2, name="ps_o")

        def psum_mm(tag="o"):
            return ps.tile([128, 512], F32, tag=tag, bufs=2, name="ps_mm")

        # ---------------- constants ----------------
        ones_full = const.tile([128, 128], BF16)
        nc.vector.memset(ones_full, 1.0)
        half_pi = const.tile([128, 1], F32)
        nc.vector.memset(half_pi, math.pi / 2)
        eps_col = const.tile([128, 1], F32)
        nc.vector.memset(eps_col, 1e-6)

        # ---------------- persistent activations ----------------
        hT = persist.tile([128, KT, N], F32)
        qTb = persist.tile([128, KT, N], BF16)
        kTb = persist.tile([128, KT, N], BF16)
        oTb = persist.tile([128, KT, N], BF16)
        # V with a ones column appended per head: cols h*65 .. h*65+64
        v_ext = persist.tile([128, 8, 6 * 65], BF16)
        for jt in range(8):
            for h in range(NH):
                nc.vector.memset(v_ext[:, jt, h * 65 + 64 : h * 65 + 65], 1.0)
        mod_cols = persist.tile([128, L, 18], F32)
        sc1_cols = persist.tile([128, L, 6], F32)  # 1+sc_a (0:3), 1+sc_m (3:6)
        fin_col = persist.tile([128, 6], F32)
        fin_sc1 = persist.tile([128, 3], F32)
        cbias_sb = persist.tile([1, D], F32)
        nc.sync.dma_start(out=cbias_sb, in_=H["cbias_in"].ap())

        # =========================================================
        # Conditioning: c_act = silu(t_emb + dt_emb + cbias)
        # =========================================================
        freqs = const.tile([128, 1], F32)
        nc.sync.dma_start(out=freqs, in_=H["freqs_in"].ap())

        def t_embed(t_h, w1_h, w2_h, b1_h, tag):
            tb = rowp.tile([128, 1], F32, tag="tb", name=f"tb_{tag}")
            nc.sync.dma_start(
                out=tb,
                in_=bass.AP(
                    tensor=t_h.ap().tensor, offset=0, ap=[[0, 128], [1, 1]]
                ),
            )
            a = rowp.tile([128, 1], F32, tag="ta", name=f"ta_{tag}")
            nc.vector.tensor_tensor(out=a, in0=freqs, in1=tb, op=OP.mult)
            ecol = rowp.tile([128, 2, 1], BF16, tag="ecol", name=f"ecol_{tag}")
            nc.scalar.activation(ecol[:, 0, :], a, AF.Sin, bias=half_pi)
            nc.scalar.activation(ecol[:, 1, :], a, AF.Sin)
            w1 = const.tile([128, 2, D], BF16, tag="tw1", name=f"w1_{tag}")
            nc.sync.dma_start(out=w1, in_=w1_h.ap())
            p1 = psum_mm()
            for k in range(2):
                nc.tensor.matmul(
                    p1[:1, :D], ecol[:, k, :], w1[:, k, :],
                    start=(k == 0), stop=(k == 1),
                )
            b1 = rowp.tile([1, D], F32, tag="b1row", bufs=1, name=f"b1_{tag}")
            nc.sync.dma_start(out=b1, in_=b1_h.ap())
            s1 = rowp.tile([1, D], F32, tag="s1row", bufs=1, name=f"s1_{tag}")
            nc.vector.tensor_tensor(out=s1, in0=p1[:1, :D], in1=b1, op=OP.add)
            s1b = rowp.tile([1, D], BF16, tag="s1brow", bufs=1, name=f"s1b_{tag}")
            nc.scalar.activation(s1b, s1, AF.Silu)
            s1c = rowp.tile([128, KT], BF16, tag="s1col", name=f"s1c_{tag}")
            row_to_col(s1b, s1c, D, BF16, f"s1_{tag}")
            w2 = const.tile([128, KT, D], BF16, tag="tw2", name=f"w2_{tag}")
            nc.sync.dma_start(out=w2, in_=w2_h.ap())
            p2 = psum_mm()
            for k in range(KT):
                nc.tensor.matmul(
                    p2[:1, :D], s1c[:, k : k + 1], w2[:, k, :],
                    start=(k == 0), stop=(k == KT - 1),
                )
            return p2

        p_t = t_embed(H["t_in"], H["t1w1_in"], H["t1w2_in"], H["t1b1_in"], "t")
        r_t = rowp.tile([1, D], F32, tag="rt", bufs=1)
        nc.scalar.copy(r_t, p_t[:1, :D])
        p_dt = t_embed(H["dt_in"], H["t2w1_in"], H["t2w2_in"], H["t2b1_in"], "dt")
        c_row = rowp.tile([1, D], F32, tag="crow", bufs=1)
        nc.vector.tensor_tensor(out=c_row, in0=r_t, in1=p_dt[:1, :D], op=OP.add)
        nc.vector.tensor_tensor(out=c_row, in0=c_row, in1=cbias_sb, op=OP.add)
        ca_row = rowp.tile([1, D], BF16, tag="carow", bufs=1)
        nc.scalar.activation(ca_row, c_row, AF.Silu)
        ca_col = persist.tile([128, KT], BF16)
        row_to_col(ca_row, ca_col, D, BF16, "ca")

        # =========================================================
        # Patch embed: hT = patch_w.T @ xp.T (+patch_b +posT)
        # =========================================================
        xph = const.tile([16, N], BF16)
        xpl = const.tile([16, N], BF16)
        pwh = const.tile([16, D], BF16)
        pwl = const.tile([16, D], BF16)
        patch_b = const.tile([128, KT], F32)
        nc.sync.dma_start(out=xph, in_=H["xpT_hi"].ap())
        nc.sync.dma_start(out=xpl, in_=H["xpT_lo"].ap())
        nc.sync.dma_start(out=pwh, in_=H["pw_hi"].ap())
        nc.sync.dma_start(out=pwl, in_=H["pw_lo"].ap())
        nc.sync.dma_start(out=patch_b, in_=H["patch_b_in"].ap())

        for mt in range(KT):
            pp = psum_s()
            for half in range(NHALF):
                sl = slice(half * 512, half * 512 + 512)
                for i, (w, x) in enumerate(((pwh, xph), (pwh, xpl), (pwl, xph))):
                    nc.tensor.matmul(
                        pp[:, sl],
                        w[:, mt * 128 : mt * 128 + 128],
                        x[:, sl],
                        start=(i == 0),
                        stop=(i == 2),
                    )
            post = resid.tile([128, N], F32, tag="res", name="post")
            nc.sync.dma_start(out=post, in_=H["posT_in"].ap()[:, mt, :])
            tmp = resid.tile([128, N], F32, tag="res", name="tmp")
            nc.vector.tensor_scalar_add(
                out=tmp, in0=pp, scalar1=patch_b[:, mt : mt + 1]
            )
            nc.vector.tensor_tensor(
                out=hT[:, mt, :], in0=tmp, in1=post, op=OP.add
            )

        if H.get("DBG"):
            dbg_h0_sb = resid.tile([128, KT, N], F32, tag="dbgbig", bufs=1, name="dbg_h0_sb")
            for mt in range(KT):
                nc.vector.tensor_copy(dbg_h0_sb[:, mt, :], hT[:, mt, :])
            nc.sync.dma_start(out=H["dbg_h0"].ap(), in_=dbg_h0_sb)

        # ---- adaLN modulation vectors (row-form matvec per layer) ----
        def emit_mod(l):
            mcol = stat.tile([128, 18], F32, tag="mcol", name=f"mcol{l}")
            nchunks = (6 * D + 511) // 512
            for ci in range(nchunks):
                cw = min(512, 6 * D - ci * 512)
                aw = wpool.tile(
                    [128, KT, 512], BF16, tag="adaln", name=f"aw{l}_{ci}"
                )
                nc.sync.dma_start(
                    out=aw[:, :, :cw],
                    in_=H["adaln_w_in"].ap()[l][:, :, ci * 512 : ci * 512 + cw],
                )
                pm = psum_mm("s")
                for k in range(KT):
                    nc.tensor.matmul(
                        pm[:1, :cw],
                        ca_col[:, k : k + 1],
                        aw[:, k, :cw],
                        start=(k == 0),
                        stop=(k == KT - 1),
                    )
                crow = rowp.tile([1, 512], F32, tag="rowbuf", name=f"mr{l}_{ci}")
                nc.scalar.copy(crow[:, :cw], pm[:1, :cw])
                row_to_col(
                    crow, mcol[:, ci * 4 : ci * 4 + cw // 128], cw, F32,
                    f"m{l}_{ci}",
                )
            ab = stat.tile([128, 18], F32, tag="abcol", name=f"ab{l}")
            nc.sync.dma_start(out=ab, in_=H["adaln_b_in"].ap()[l])
            nc.vector.tensor_tensor(
                out=mod_cols[:, l, :], in0=mcol, in1=ab, op=OP.add
            )
            # 1 + sc_a (cols 3:6), 1 + sc_m (cols 12:15)
            nc.vector.tensor_scalar_add(
                out=sc1_cols[:, l, 0:3], in0=mod_cols[:, l, 3:6], scalar1=1.0
            )
            nc.vector.tensor_scalar_add(
                out=sc1_cols[:, l, 3:6], in0=mod_cols[:, l, 12:15], scalar1=1.0
            )

        def emit_fin_mod():
            fcol_t = stat.tile([128, 6], F32, tag="mcol", name="fcol_t")
            for ci in range(2):
                cw = min(512, 2 * D - ci * 512)
                fw = wpool.tile(
                    [128, KT, 512], BF16, tag="adaln", name=f"fw{ci}"
                )
                nc.sync.dma_start(
                    out=fw[:, :, :cw],
                    in_=H["finmw_in"].ap()[:, :, ci * 512 : ci * 512 + cw],
                )
                pm = psum_mm("s")
                for k in range(KT):
                    nc.tensor.matmul(
                        pm[:1, :cw],
                        ca_col[:, k : k + 1],
                        fw[:, k, :cw],
                        start=(k == 0),
                        stop=(k == KT - 1),
                    )
                crow = rowp.tile([1, 512], F32, tag="rowbuf", name=f"fr{ci}")
                nc.scalar.copy(crow[:, :cw], pm[:1, :cw])
                row_to_col(
                    crow, fcol_t[:, ci * 4 : ci * 4 + cw // 128], cw, F32,
                    f"f{ci}",
                )
            fb = stat.tile([128, 6], F32, tag="abcol", name="fb")
            nc.sync.dma_start(out=fb, in_=H["finmb_in"].ap())
            nc.vector.tensor_tensor(out=fin_col, in0=fcol_t, in1=fb, op=OP.add)
            nc.vector.tensor_scalar_add(
                out=fin_sc1, in0=fin_col[:, 3:6], scalar1=1.0
            )

        emit_mod(0)
        if n_layers > 1:
            emit_mod(1)

        # =========================================================
        # LayerNorm -> modulated xm (T-domain)
        # =========================================================
        def layernorm_mod(xmTb, gamma, beta, xmLo=None, interleave=None):
            """xmTb[:, k, :] = LN_token(hT)*gamma + beta (bf16 out).

            meanB/sqB are produced directly as [128, 512] broadcast sums via
            rank-1 ones matmuls accumulated over the 3 k-tiles.
            """
            hTb = act16.tile([128, KT, N], BF16, tag="a16", bufs=3, name="hTb")
            sq = act16.tile([128, KT, N], BF16, tag="a16", bufs=3, name="sq")
            for k in range(KT):
                nc.vector.tensor_copy(hTb[:, k, :], hT[:, k, :])
                nc.vector.tensor_tensor(
                    out=sq[:, k, :], in0=hTb[:, k, :], in1=hTb[:, k, :],
                    op=OP.mult,
                )
            pM = [None, None]
            pQ = [None, None]
            for half in range(NHALF):
                sl = slice(half * 512, half * 512 + 512)
                pM[half] = psum_s([128, 512])
                for k in range(KT):
                    nc.tensor.matmul(
                        pM[half], ones_full, hTb[:, k, sl],
                        start=(k == 0), stop=(k == KT - 1),
                    )
                pQ[half] = psum_s([128, 512])
                for k in range(KT):
                    nc.tensor.matmul(
                        pQ[half], ones_full, sq[:, k, sl],
                        start=(k == 0), stop=(k == KT - 1),
                    )
                if half == 0 and interleave is not None:
                    interleave()
                msq = stat.tile([128, 512], F32, tag="statA", bufs=2, name="msq")
                nc.scalar.activation(msq, pM[half], AF.Square, scale=1.0 / D)
                var = stat.tile([128, 512], F32, tag="statA", bufs=2, name="var")
                nc.vector.scalar_tensor_tensor(
                    out=var, in0=pQ[half], scalar=1.0 / D, in1=msq,
                    op0=OP.mult, op1=OP.subtract,
                )
                sd = stat.tile([128, 512], F32, tag="statA", bufs=2, name="sd")
                nc.scalar.activation(sd, var, AF.Sqrt, bias=eps_col)
                rstd = stat.tile([128, 512], F32, tag="rstd", name="rstd")
                rscr = stat.tile([128, 512], F32, tag="statB", bufs=2, name="rscr")
                with nc.allow_low_precision(reason="rstd at ~2 ULP via NR"):
                    nc.vector.reciprocal_approx_accurate(
                        out=rstd, in_=sd, scratch=rscr
                    )
                for k in range(KT):
                    d_ = stat.tile([128, 512], F32, tag="statB", bufs=2, name="d_")
                    nc.vector.scalar_tensor_tensor(
                        out=d_, in0=pM[half], scalar=-1.0 / D, in1=hT[:, k, sl],
                        op0=OP.mult, op1=OP.add,
                    )
                    e_ = stat.tile([128, 512], F32, tag="statB", bufs=2, name="e_")
                    nc.vector.tensor_tensor(out=e_, in0=d_, in1=rstd, op=OP.mult)
                    if xmLo is None:
                        nc.vector.tensor_scalar(
                            out=xmTb[:, k, sl], in0=e_,
                            scalar1=gamma[:, k : k + 1], scalar2=beta[:, k : k + 1],
                            op0=OP.mult, op1=OP.add,
                        )
                    else:
                        xf = stat.tile(
                            [128, 512], F32, tag="statB", bufs=2, name="xf"
                        )
                        nc.vector.tensor_scalar(
                            out=xf, in0=e_,
                            scalar1=gamma[:, k : k + 1], scalar2=beta[:, k : k + 1],
                            op0=OP.mult, op1=OP.add,
                        )
                        nc.vector.tensor_copy(xmTb[:, k, sl], xf)
                        nc.vector.scalar_tensor_tensor(
                            out=xmLo[:, k, sl], in0=xmTb[:, k, sl], scalar=-1.0,
                            in1=xf, op0=OP.mult, op1=OP.add,
                        )

        # =========================================================
        # Transformer layers
        # =========================================================
        for l in range(n_layers):
            mc = mod_cols[:, l, :]
            sc1 = sc1_cols[:, l, :]
            # ---- LN1 + modulation ----
            xmTb = act16.tile([128, KT, N], BF16, tag="a16", bufs=3, name="xmTb")
            layernorm_mod(xmTb, sc1[:, 0:3], mc[:, 0:3])

            # ---- qkv ----
            qw = wpool.tile([128, KT, D], BF16, tag="qw", name=f"qw{l}")
            kw = wpool.tile([128, KT, D], BF16, tag="kw", name=f"kw{l}")
            vw = wpool.tile([128, KT, D], BF16, tag="vw", name=f"vw{l}")
            nc.sync.dma_start(out=qw, in_=H["qw_in"].ap()[l])
            nc.sync.dma_start(out=kw, in_=H["kw_in"].ap()[l])
            nc.sync.dma_start(out=vw, in_=H["vw_in"].ap()[l])
            qb = stat.tile([128, KT], F32, tag="bcol", bufs=4, name=f"qb{l}")
            kb = stat.tile([128, KT], F32, tag="bcol", bufs=4, name=f"kb{l}")
            nc.sync.dma_start(out=qb, in_=H["qb_in"].ap()[l])
            nc.sync.dma_start(out=kb, in_=H["kb_in"].ap()[l])
            vbB = stat.tile([128, D], F32, tag="vbB", bufs=1, name=f"vbB{l}")
            nc.sync.dma_start(
                out=vbB,
                in_=bass.AP(
                    tensor=H["vb_row_in"].ap().tensor,
                    offset=l * D,
                    ap=[[0, 128], [1, D]],
                ),
            )

            for mt in range(KT):
                pq = psum_s()
                for half in range(NHALF):
                    sl = slice(half * 512, half * 512 + 512)
                    for k in range(KT):
                        nc.tensor.matmul(
                            pq[:, sl], qw[:, k, mt * 128 : mt * 128 + 128],
                            xmTb[:, k, sl],
                            start=(k == 0), stop=(k == KT - 1),
                        )
                nc.vector.tensor_scalar(
                    out=qTb[:, mt, :], in0=pq,
                    scalar1=qb[:, mt : mt + 1], scalar2=1.0 / HD,
                    op0=OP.add, op1=OP.mult,
                )
                pk = psum_s()
                for half in range(NHALF):
                    sl = slice(half * 512, half * 512 + 512)
                    for k in range(KT):
                        nc.tensor.matmul(
                            pk[:, sl], kw[:, k, mt * 128 : mt * 128 + 128],
                            xmTb[:, k, sl],
                            start=(k == 0), stop=(k == KT - 1),
                        )
                nc.vector.tensor_scalar_add(
                    out=kTb[:, mt, :], in0=pk, scalar1=kb[:, mt : mt + 1]
                )
            # v in normal layout [token, d] with bias, strided into v_ext
            for jt in range(8):
                pv = psum_mm()
                jsl = slice(jt * 128, jt * 128 + 128)
                for k in range(KT):
                    nc.tensor.matmul(
                        pv[:, :D], xmTb[:, k, jsl], vw[:, k, :],
                        start=(k == 0), stop=(k == KT - 1),
                    )
                vdst = bass.AP(
                    tensor=v_ext.tensor,
                    offset=v_ext.offset + jt * (6 * 65),
                    ap=[list(v_ext.ap[0]), [65, 6], [1, 64]],
                )
                pv_v = bass.AP(
                    tensor=pv.tensor, offset=pv.offset,
                    ap=[list(pv.ap[0]), [64, 6], [1, 64]],
                )
                vb_v = bass.AP(
                    tensor=vbB.tensor, offset=vbB.offset,
                    ap=[list(vbB.ap[0]), [64, 6], [1, 64]],
                )
                nc.vector.tensor_tensor(out=vdst, in0=pv_v, in1=vb_v, op=OP.add)

            # ---- attention: process heads in pairs (PE row-tiling:
            # even head occupies rows 0-63, odd head rows 64-127, their
            # score matmuls run concurrently on the array) ----
            for hp in range(NH // 2):
                mt = hp  # head pair hp = heads (2hp, 2hp+1); mt = hp
                po_oA = psum_o()
                po_oB = psum_o()
                for jt in range(8):
                    jsl = slice(jt * 128, jt * 128 + 128)
                    ps_sA = psum_s()
                    ps_sB = psum_s()
                    for half in range(NHALF):
                        sl = slice(half * 512, half * 512 + 512)
                        nc.tensor.matmul(
                            ps_sA[:, sl],
                            kTb[0:64, mt, jsl],
                            qTb[0:64, mt, sl],
                            start=True, stop=True,
                        )
                        nc.tensor.matmul(
                            ps_sB[:, sl],
                            kTb[64:128, mt, jsl],
                            qTb[64:128, mt, sl],
                            start=True, stop=True,
                        )
                    expA = expp.tile([128, N], BF16, tag="expT", name="expA")
                    nc.scalar.activation(expA, ps_sA, AF.Exp)
                    expB = expp.tile([128, N], BF16, tag="expT", name="expB")
                    if jt % 2 == 0:
                        # Offload to VectorE: exp(s) ~= (1+s/2)^2 for |s|<=0.2
                        # (error ~s^2/4 on unnormalized weights; diluted to
                        # ~1e-4 by softmax normalization + the residual gate).
                        ab_ = expp.tile([128, N], BF16, tag="expT", name="ab_")
                        nc.vector.tensor_scalar(
                            out=ab_, in0=ps_sB, scalar1=0.5, scalar2=1.0,
                            op0=OP.mult, op1=OP.add,
                        )
                        nc.vector.tensor_tensor(
                            out=expB, in0=ab_, in1=ab_, op=OP.mult
                        )
                    else:
                        nc.scalar.activation(expB, ps_sB, AF.Exp)
                    for half in range(NHALF):
                        sl = slice(half * 512, half * 512 + 512)
                        nc.tensor.matmul(
                            po_oA[:, sl],
                            v_ext[:, jt, (2 * hp) * 65 : (2 * hp) * 65 + 65],
                            expA[:, sl],
                            start=(jt == 0), stop=(jt == 7),
                        )
                        nc.tensor.matmul(
                            po_oB[:, sl],
                            v_ext[:, jt, (2 * hp + 1) * 65 : (2 * hp + 1) * 65 + 65],
                            expB[:, sl],
                            start=(jt == 0), stop=(jt == 7),
                        )
                # Batched 1/Z for the pair: rows 0-63 broadcast Z_A, rows
                # 64-127 broadcast Z_B; one reciprocal covers both heads.
                zrowA = rowp.tile([1, N], F32, tag="zrow", bufs=1, name="zrowA")
                nc.scalar.copy(zrowA, po_oA[64:65, :])
                zrowB = rowp.tile([1, N], F32, tag="zrowB", bufs=1, name="zrowB")
                nc.vector.tensor_copy(zrowB, po_oB[64:65, :])
                zscrA = dramp.tile([N], F32, tag="zscr", name="zscrA")
                nc.sync.dma_start(out=zscrA, in_=zrowA[:1, :])
                zscrB = dramp.tile([N], F32, tag="zscrB", name="zscrB")
                nc.sync.dma_start(out=zscrB, in_=zrowB[:1, :])
                zbB = expp.tile([128, N], F32, tag="zbB", bufs=1, name="zbB")
                nc.sync.dma_start(
                    out=zbB[0:64, :],
                    in_=bass.AP(
                        tensor=zscrA.tensor, offset=zscrA.offset,
                        ap=[[0, 64], [1, N]],
                    ),
                )
                nc.sync.dma_start(
                    out=zbB[64:128, :],
                    in_=bass.AP(
                        tensor=zscrB.tensor, offset=zscrB.offset,
                        ap=[[0, 64], [1, N]],
                    ),
                )
                zrec = expp.tile([128, N], F32, tag="zrec", bufs=1, name="zrec")
                with nc.allow_low_precision(reason="1/Z at 18 bits"):
                    nc.vector.reciprocal_approx_fast(out=zrec, in_=zbB)
                nc.vector.tensor_tensor(
                    out=oTb[0:64, mt, :],
                    in0=po_oA[0:64, :], in1=zrec[0:64, :], op=OP.mult,
                )
                nc.vector.tensor_tensor(
                    out=oTb[64:128, mt, :],
                    in0=po_oB[0:64, :], in1=zrec[64:128, :], op=OP.mult,
                )

            # adaLN matvecs for layer l+2 fill the attention-tail PE bubble
            if l + 2 < n_layers:
                emit_mod(l + 2)
            elif l + 2 == n_layers:
                emit_fin_mod()

            # ---- o-proj + residual ----
            # k=0,1 (head pairs 0/1, already finished) are emitted first so
            # they run while the last pair's 1/Z chain completes; the k=2
            # matmuls join as pair 2 lands.
            ow = wpool.tile([128, KT, D], BF16, tag="ow", name=f"ow{l}")
            nc.sync.dma_start(out=ow, in_=H["ow_in"].ap()[l])
            ob = stat.tile([128, KT], F32, tag="bcol", bufs=4, name=f"ob{l}")
            nc.sync.dma_start(out=ob, in_=H["ob_in"].ap()[l])

            def oproj_mm(pr, mt, k, first, last):
                for half in range(NHALF):
                    sl = slice(half * 512, half * 512 + 512)
                    nc.tensor.matmul(
                        pr[:, sl], ow[:, k, mt * 128 : mt * 128 + 128],
                        oTb[:, k, sl], start=first, stop=last,
                    )

            def oproj_evac(pr, mt):
                tmp = resid.tile([128, N], F32, tag="res", name="tmpo")
                nc.vector.tensor_scalar(
                    out=tmp, in0=pr,
                    scalar1=ob[:, mt : mt + 1], scalar2=mc[:, 6 + mt : 7 + mt],
                    op0=OP.add, op1=OP.mult,
                )
                nc.vector.tensor_tensor(
                    out=hT[:, mt, :], in0=hT[:, mt, :], in1=tmp, op=OP.add
                )

            pr0 = psum_s()
            for k in range(2):
                oproj_mm(pr0, 0, k, k == 0, False)
            pr1 = psum_s()
            for k in range(2):
                oproj_mm(pr1, 1, k, k == 0, False)
            oproj_mm(pr0, 0, 2, False, True)
            oproj_evac(pr0, 0)
            pr2 = psum_s()
            for k in range(KT):
                oproj_mm(pr2, 2, k, k == 0, k == KT - 1)
            oproj_mm(pr1, 1, 2, False, True)
            oproj_evac(pr1, 1)
            oproj_evac(pr2, 2)

            # ---- LN2 + MLP ----
            xm2Tb = act16.tile([128, KT, N], BF16, tag="a16", bufs=3, name="xm2Tb")
            layernorm_mod(xm2Tb, sc1[:, 3:6], mc[:, 9:12])

            mw1 = wpool.tile([128, KT, MLP], BF16, tag="mw1", name=f"mw1{l}")
            mw2 = wpool.tile([128, MT1, D], BF16, tag="mw2", name=f"mw2{l}")
            nc.sync.dma_start(out=mw1, in_=H["mw1_in"].ap()[l])
            nc.sync.dma_start(out=mw2, in_=H["mw2_in"].ap()[l])
            mb1 = stat.tile([128, MT1], F32, tag="mb1col", name=f"mb1{l}")
            mb2 = stat.tile([128, KT], F32, tag="bcol", bufs=4, name=f"mb2{l}")
            nc.sync.dma_start(out=mb1, in_=H["mb1_in"].ap()[l])
            nc.sync.dma_start(out=mb2, in_=H["mb2_in"].ap()[l])

            for half in range(NHALF):
                sl = slice(half * 512, half * 512 + 512)
                gTb = act16.tile(
                    [128, MT1, 512], BF16, tag="gTb", name="gTb"
                )
                for mt in range(MT1):
                    pg = psum_mm()
                    for k in range(KT):
                        nc.tensor.matmul(
                            pg, mw1[:, k, mt * 128 : mt * 128 + 128],
                            xm2Tb[:, k, sl],
                            start=(k == 0), stop=(k == KT - 1),
                        )
                    nc.scalar.activation(
                        gTb[:, mt, :], pg, AF.Gelu, bias=mb1[:, mt : mt + 1]
                    )
                for mt in range(KT):
                    pf = psum_mm()
                    for k in range(MT1):
                        nc.tensor.matmul(
                            pf, mw2[:, k, mt * 128 : mt * 128 + 128],
                            gTb[:, k, :],
                            start=(k == 0), stop=(k == MT1 - 1),
                        )
                    tmp2 = resid.tile([128, 512], F32, tag="resh", name="tmpm")
                    nc.vector.tensor_scalar(
                        out=tmp2, in0=pf,
                        scalar1=mb2[:, mt : mt + 1],
                        scalar2=mc[:, 15 + mt : 16 + mt],
                        op0=OP.add, op1=OP.mult,
                    )
                    nc.vector.tensor_tensor(
                        out=hT[:, mt, sl], in0=hT[:, mt, sl], in1=tmp2, op=OP.add
                    )

        # =========================================================
        # Final layer
        # =========================================================
        if n_layers < 2:
            emit_fin_mod()
        xmF = act16.tile([128, KT, N], BF16, tag="a16", bufs=3, name="xmF")
        xmFlo = act16.tile([128, KT, N], BF16, tag="a16", bufs=3, name="xmFlo")
        layernorm_mod(xmF, fin_sc1, fin_col[:, 0:3], xmLo=xmFlo)
        if H.get("DBG"):
            nc.sync.dma_start(out=H["dbg_fincol"].ap(), in_=fin_col)
            dbg_xm_sb = resid.tile([128, KT, N], F32, tag="dbgbig", bufs=1, name="dbg_xm_sb")
            for mt in range(KT):
                nc.vector.tensor_copy(dbg_xm_sb[:, mt, :], xmF[:, mt, :])
            nc.sync.dma_start(out=H["dbg_xmF"].ap(), in_=dbg_xm_sb)
            dbg_ca_sb = resid.tile([128, KT], F32, tag="dbgca", bufs=1, name="dbg_ca_sb")
            nc.vector.tensor_copy(dbg_ca_sb, ca_col)
            nc.sync.dma_start(out=H["dbg_ca"].ap(), in_=dbg_ca_sb)
        # hi/lo split of xmF for the precision-critical final matmul:
        # xmF is already bf16; recompute hi=xmF (bf16) and lo = fp32(xm) - hi.
        # We recompute the fp32 xm into a temp to form lo.
        # Cheaper: lo = (e2*gamma+beta) - xmF computed per tile below.
        fpwh = const.tile([128, KT, 16], BF16)
        fpwl = const.tile([128, KT, 16], BF16)
        fpb = const.tile([16, 1], F32)
        nc.sync.dma_start(out=fpwh, in_=H["fpw_hi"].ap())
        nc.sync.dma_start(out=fpwl, in_=H["fpw_lo"].ap())
        nc.sync.dma_start(out=fpb, in_=H["fpb_in"].ap())

        pout = psum_mm()
        pout2 = psum_mm()
        pps = [pout, pout2]
        for half in range(NHALF):
            sl = slice(half * 512, half * 512 + 512)
            mms = []
            for k in range(KT):
                mms.append((fpwh[:, k, :], xmF[:, k, sl]))
                mms.append((fpwl[:, k, :], xmF[:, k, sl]))
                mms.append((fpwh[:, k, :], xmFlo[:, k, sl]))
            for i, (wv, xv) in enumerate(mms):
                nc.tensor.matmul(
                    pps[half][:16, :], wv, xv,
                    start=(i == 0), stop=(i == len(mms) - 1),
                )
        out_sb = resid.tile([16, N], F32, tag="outsb", bufs=1, name="out_sb")
        for half in range(NHALF):
            sl = slice(half * 512, half * 512 + 512)
            nc.vector.tensor_scalar_add(
                out=out_sb[:, sl], in0=pps[half][:16, :], scalar1=fpb
            )
        nc.sync.dma_start(out=H["outT"].ap(), in_=out_sb)


# =================================================================
# Host side
# =================================================================
_BUILD_CACHE = {}


def _get_module(n_layers=L):
    if n_layers not in _BUILD_CACHE:
        _register_profile_hook()
        _BUILD_CACHE[n_layers] = build_module(n_layers)
    return _BUILD_CACHE[n_layers]


def _shuf_w(w):
    """[Din, Dout] -> [128, Din//128, Dout], partition-contiguous."""
    din = w.shape[0]
    return np.ascontiguousarray(
        w.reshape(din // 128, 128, -1).transpose(1, 0, 2)
    )


def _col(v):
    """[D] -> [128, D//128] column layout."""
    return np.ascontiguousarray(v.reshape(-1, 128).T)


def _bf(x):
    return np.asarray(x, np.float32).astype(ml_dtypes.bfloat16)


def prepare_inputs(inputs, n_layers=L):
    """Build the 8 per-core in_maps from the full input dict."""
    ii = {k: np.asarray(v) for k, v in inputs.items()}
    x = ii["x"].astype(np.float32)
    t = ii["t"].astype(np.float32)
    dt = ii["dt"].astype(np.float32)
    y = ii["y"].astype(np.int64)
    pos = ii["pos"].astype(np.float32)

    shared = {}
    shared["posT"] = _shuf_w(np.ascontiguousarray(pos.T))
    pw = ii["patch_w"].astype(np.float32)
    pwh = pw.astype(ml_dtypes.bfloat16)
    shared["pw_hi"] = pwh
    shared["pw_lo"] = (pw - pwh.astype(np.float32)).astype(ml_dtypes.bfloat16)
    shared["patch_b"] = _col(ii["patch_b"].astype(np.float32))
    shared["t1w1"] = _shuf_w(_bf(ii["t1_w1"]))
    shared["t1w2"] = _shuf_w(_bf(ii["t1_w2"]))
    shared["t2w1"] = _shuf_w(_bf(ii["t2_w1"]))
    shared["t2w2"] = _shuf_w(_bf(ii["t2_w2"]))
    shared["t1b1"] = ii["t1_b1"].astype(np.float32).reshape(1, D)
    shared["t2b1"] = ii["t2_b1"].astype(np.float32).reshape(1, D)
    half = FREQ // 2
    shared["freqs"] = np.exp(
        -math.log(10000.0) * np.arange(half, dtype=np.float64) / half
    ).astype(np.float32).reshape(half, 1)
    shared["adaln_w"] = np.stack([_shuf_w(_bf(ii["adaln_w"][l])) for l in range(L)])
    shared["adaln_b"] = np.stack(
        [_col(ii["adaln_b"][l].astype(np.float32)) for l in range(L)]
    )
    for nm, src in (("qw", "q_w"), ("kw", "k_w"), ("vw", "v_w"), ("ow", "o_w")):
        shared[nm] = np.stack([_shuf_w(_bf(ii[src][l])) for l in range(L)])
    for nm, src in (("qb", "q_b"), ("kb", "k_b"), ("ob", "o_b"), ("mb2", "m_b2")):
        shared[nm] = np.stack(
            [_col(ii[src][l].astype(np.float32)) for l in range(L)]
        )
    shared["vb_row"] = ii["v_b"].astype(np.float32).reshape(L, 1, D)
    shared["mw1"] = np.stack([_shuf_w(_bf(ii["m_w1"][l])) for l in range(L)])
    shared["mw2"] = np.stack([_shuf_w(_bf(ii["m_w2"][l])) for l in range(L)])
    shared["mb1"] = np.stack(
        [_col(ii["m_b1"][l].astype(np.float32)) for l in range(L)]
    )
    shared["finmw"] = _shuf_w(_bf(ii["fin_mw"]))
    shared["finmb"] = _col(ii["fin_mb"].astype(np.float32))
    fpw = _shuf_w(ii["fin_pw"].astype(np.float32))
    fpwh = fpw.astype(ml_dtypes.bfloat16)
    shared["fpw_hi"] = fpwh
    shared["fpw_lo"] = (fpw - fpwh.astype(np.float32)).astype(ml_dtypes.bfloat16)
    shared["fpb"] = ii["fin_pb"].astype(np.float32).reshape(16, 1)

    label_emb = ii["label_emb"].astype(np.float32)
    cb_common = (ii["t1_b2"].astype(np.float32) + ii["t2_b2"].astype(np.float32))

    in_maps = []
    for b in range(B):
        m = dict(shared)
        xp = (
            x[b]
            .reshape(CIN, HP, P, HP, P)
            .transpose(1, 3, 0, 2, 4)
            .reshape(N, CIN * P * P)
        )
        xpT = np.ascontiguousarray(xp.T)
        xph = xpT.astype(ml_dtypes.bfloat16)
        m["xpT_hi"] = xph
        m["xpT_lo"] = (xpT - xph.astype(np.float32)).astype(ml_dtypes.bfloat16)
        m["t_in"] = t[b].reshape(1, 1)
        m["dt_in"] = dt[b].reshape(1, 1)
        m["cbias"] = (cb_common + label_emb[int(y[b])]).reshape(1, D)
        in_maps.append(m)
    return in_maps


def assemble_output(results):
    out = np.empty((B, COUT, IMG, IMG), np.float32)
    for b in range(B):
        tok = results[b]["outT"].T  # [N, 16]
        out[b] = (
            tok.reshape(HP, HP, P, P, COUT)
            .transpose(4, 0, 2, 1, 3)
            .reshape(COUT, IMG, IMG)
        )
    return out


def run(inputs, n_layers=L, trace=False, sim=False):
    nc = _get_module(n_layers)
    in_maps = prepare_inputs(inputs, n_layers)
    if sim:
        from concourse.bass_interp import CoreSim

        s = CoreSim(nc, trace=False)
        for k, v in in_maps[0].items():
            s.tensor(k)[:] = v
        s.simulate()
        names = ["outT"]
        if os.environ.get("DIT_DBG", "0") == "1":
            names += ["dbg_h0", "dbg_fincol", "dbg_xmF", "dbg_ca"]
        results = [
            {n: np.array(s.tensor(n)) for n in names} for _ in range(B)
        ]
        return results, None
    res = run_bass_kernel_spmd(
        nc, in_maps, core_ids=list(range(B)), trace=trace
    )
    return res.results, res


def kernel(**inputs):
    results, _ = run(inputs, L, trace=False, sim=False)
    return assemble_output(results)


# revision 21
# speedup vs baseline: 1.0401x; 1.0401x over previous
"""DiT forward kernel for 8 Trainium2 NeuronCores.

Sharding: data-parallel over batch B=8 (one batch element per core).
Per-core layout: activations live transposed ("T-domain") in SBUF as
[D on partitions, tokens on free]. All projection matmuls then use the
weight matrix as stored ([Din, Dout]) for lhsT and the activation as the
moving operand (or vice versa), so no activation transposes are needed.

Attention: scores are computed in [j, i] layout (lhsT = kT slice,
rhs = qT), exp on ScalarE with no max-subtraction (|s| <= ~0.16 by
construction), the softmax denominator comes free from a ones-column
appended to V, and normalization is applied after accumulating
o = exp(s) @ V via a PE row-broadcast of 1/Z.

All matmuls are bf16 (PSUM accumulates fp32). The two precision-critical
matmuls (patch embed, final projection) use a bf16 hi+lo split (~fp32
quality). The residual stream stays fp32 in SBUF.
"""

import math
import os
import sys
import types

sys.path.insert(0, "/opt/trn_rl_repo")

import numpy as np
import ml_dtypes

import concourse.bass as bass
import concourse.tile as tile
from concourse import bacc, mybir
from concourse.bass_utils import run_bass_kernel_spmd

F32 = mybir.dt.float32
BF16 = mybir.dt.bfloat16
AF = mybir.ActivationFunctionType
OP = mybir.AluOpType

B = 8
CIN = 4
IMG = 64
P = 2
D = 384
NH = 6
L = 12
MLP = 4 * D
NCLS = 1000
FREQ = 256
COUT = 4
HP = IMG // P
N = HP * HP  # 1024 tokens
HD = D // NH  # 64
KT = D // 128  # 3 k-tiles of the model dim
MT1 = MLP // 128  # 12 m-tiles of the MLP dim
NHALF = N // 512  # 2 column halves of 512 tokens


def _register_profile_hook():
    """The stub antenv lacks axon_hooks; register the NTFF hook ourselves."""
    if "antenv.axon_hooks" in sys.modules:
        return
    try:
        import antenv
        from trn_agent_boot.trn_boot import _ntff_profile_via_ctypes

        mod = types.ModuleType("antenv.axon_hooks")
        mod._hook = _ntff_profile_via_ctypes("/opt/axon/libaxon_pjrt.so")
        mod.get_axon_ntff_profile_hook = lambda: mod._hook
        mod.set_axon_ntff_profile_hook = lambda h: setattr(mod, "_hook", h)
        sys.modules["antenv.axon_hooks"] = mod
        antenv.axon_hooks = mod
    except Exception:
        pass


def build_module(n_layers=L):
    """Emit the full per-core BIR module. Returns the Bacc."""
    nc = bacc.Bacc(None, target_bir_lowering=False)

    def din(name, shape, dtype=F32):
        return nc.declare_dram_parameter(name, list(shape), dtype, isOutput=False)

    # ---- per-core inputs ----
    xpT_hi = din("xpT_hi", [16, N], BF16)
    xpT_lo = din("xpT_lo", [16, N], BF16)
    t_in = din("t_in", [1, 1])
    dt_in = din("dt_in", [1, 1])
    cbias_in = din("cbias", [1, D])  # t1_b2 + t2_b2 + label_emb[y]

    # ---- shared weights (host pre-shuffled) ----
    posT_in = din("posT", [128, KT, N])
    pw_hi = din("pw_hi", [16, D], BF16)
    pw_lo = din("pw_lo", [16, D], BF16)
    patch_b_in = din("patch_b", [128, KT])
    t1w1_in = din("t1w1", [128, 2, D], BF16)
    t1w2_in = din("t1w2", [128, KT, D], BF16)
    t2w1_in = din("t2w1", [128, 2, D], BF16)
    t2w2_in = din("t2w2", [128, KT, D], BF16)
    t1b1_in = din("t1b1", [1, D])
    t2b1_in = din("t2b1", [1, D])
    freqs_in = din("freqs", [128, 1])
    adaln_w_in = din("adaln_w", [L, 128, KT, 6 * D], BF16)
    adaln_b_in = din("adaln_b", [L, 128, 18])
    qw_in = din("qw", [L, 128, KT, D], BF16)
    kw_in = din("kw", [L, 128, KT, D], BF16)
    vw_in = din("vw", [L, 128, KT, D], BF16)
    ow_in = din("ow", [L, 128, KT, D], BF16)
    qb_in = din("qb", [L, 128, KT])
    kb_in = din("kb", [L, 128, KT])
    vb_row_in = din("vb_row", [L, 1, D])
    ob_in = din("ob", [L, 128, KT])
    mw1_in = din("mw1", [L, 128, KT, MLP], BF16)
    mw2_in = din("mw2", [L, 128, MT1, D], BF16)
    mb1_in = din("mb1", [L, 128, MT1])
    mb2_in = din("mb2", [L, 128, KT])
    finmw_in = din("finmw", [128, KT, 2 * D], BF16)
    finmb_in = din("finmb", [128, 6])
    fpw_hi = din("fpw_hi", [128, KT, 16], BF16)
    fpw_lo = din("fpw_lo", [128, KT, 16], BF16)
    fpb_in = din("fpb", [16, 1])

    outT = nc.declare_dram_parameter("outT", [16, N], F32, isOutput=True)
    DBG = os.environ.get("DIT_DBG", "0") == "1"
    if DBG:
        dbg_h0 = nc.declare_dram_parameter("dbg_h0", [128, KT, N], F32, isOutput=True)
        dbg_fincol = nc.declare_dram_parameter("dbg_fincol", [128, 6], F32, isOutput=True)
        dbg_xmF = nc.declare_dram_parameter("dbg_xmF", [128, KT, N], F32, isOutput=True)
        dbg_ca = nc.declare_dram_parameter("dbg_ca", [128, KT], F32, isOutput=True)

    with tile.TileContext(nc) as tc:
        _emit(nc, tc, locals(), n_layers)
    nc.compile()
    return nc


def _emit(nc, tc, H, n_layers):
    import contextlib

    ctx = contextlib.ExitStack()
    with ctx:
        # ---------------- pools ----------------
        const = ctx.enter_context(tc.tile_pool(name="const", bufs=1))
        persist = ctx.enter_context(tc.tile_pool(name="persist", bufs=1))
        wpool = ctx.enter_context(tc.tile_pool(name="wpool", bufs=2))
        act16 = ctx.enter_context(tc.tile_pool(name="act16", bufs=1))
        stat = ctx.enter_context(tc.tile_pool(name="stat", bufs=2))
        rowp = ctx.enter_context(tc.tile_pool(name="rowp", bufs=2))
        expp = ctx.enter_context(tc.tile_pool(name="expp", bufs=3))
        resid = ctx.enter_context(tc.tile_pool(name="resid", bufs=2))
        ps = ctx.enter_context(tc.tile_pool(name="ps", bufs=1, space="PSUM"))
        dramp = ctx.enter_context(tc.tile_pool(name="dramp", bufs=2, space="DRAM"))

        def row_to_col(row, col, nelem, dtype, nm):
            """[1, nelem] sbuf row -> [128, nelem//128] sbuf column layout."""
            scr = dramp.tile([nelem], dtype, tag="scr", name=f"scr_{nm}")
            nc.sync.dma_start(out=scr, in_=row[:1, :nelem])
            nc.sync.dma_start(
                out=col,
                in_=bass.AP(
                    tensor=scr.tensor, offset=scr.offset,
                    ap=[[1, 128], [128, nelem // 128]],
                ),
            )

        def psum_s(shape=None):
            return ps.tile(shape or [128, N], F32, tag="s", bufs=2, name="ps_s")

        def psum_o(shape=None):
            return ps.tile(shape or [65, N], F32, tag="o", bufs=2, name="ps_o")

        def psum_mm(tag="o"):
            return ps.tile([128, 512], F32, tag=tag, bufs=2, name="ps_mm")

        # ---------------- constants ----------------
        ones_full = const.tile([128, 128], BF16)
        nc.vector.memset(ones_full, 1.0)
        half_pi = const.tile([128, 1], F32)
        nc.vector.memset(half_pi, math.pi / 2)
        eps_col = const.tile([128, 1], F32)
        nc.vector.memset(eps_col, 1e-6)

        # ---------------- persistent activations ----------------
        hT = persist.tile([128, KT, N], F32)
        qTb = persist.tile([128, KT, N], BF16)
        kTb = persist.tile([128, KT, N], BF16)
        oTb = persist.tile([128, KT, N], BF16)
        # V with a ones column appended per head: cols h*65 .. h*65+64
        v_ext = persist.tile([128, 8, 6 * 65], BF16)
        for jt in range(8):
            for h in range(NH):
                nc.vector.memset(v_ext[:, jt, h * 65 + 64 : h * 65 + 65], 1.0)
        mod_cols = persist.tile([128, L, 18], F32)
        sc1_cols = persist.tile([128, L, 6], F32)  # 1+sc_a (0:3), 1+sc_m (3:6)
        fin_col = persist.tile([128, 6], F32)
        fin_sc1 = persist.tile([128, 3], F32)
        cbias_sb = persist.tile([1, D], F32)
        nc.sync.dma_start(out=cbias_sb, in_=H["cbias_in"].ap())

        # =========================================================
        # Conditioning: c_act = silu(t_emb + dt_emb + cbias)
        # =========================================================
        freqs = const.tile([128, 1], F32)
        nc.sync.dma_start(out=freqs, in_=H["freqs_in"].ap())

        def t_embed(t_h, w1_h, w2_h, b1_h, tag):
            tb = rowp.tile([128, 1], F32, tag="tb", name=f"tb_{tag}")
            nc.sync.dma_start(
                out=tb,
                in_=bass.AP(
                    tensor=t_h.ap().tensor, offset=0, ap=[[0, 128], [1, 1]]
                ),
            )
            a = rowp.tile([128, 1], F32, tag="ta", name=f"ta_{tag}")
            nc.vector.tensor_tensor(out=a, in0=freqs, in1=tb, op=OP.mult)
            ecol = rowp.tile([128, 2, 1], BF16, tag="ecol", name=f"ecol_{tag}")
            nc.scalar.activation(ecol[:, 0, :], a, AF.Sin, bias=half_pi)
            nc.scalar.activation(ecol[:, 1, :], a, AF.Sin)
            w1 = const.tile([128, 2, D], BF16, tag="tw1", name=f"w1_{tag}")
            nc.sync.dma_start(out=w1, in_=w1_h.ap())
            p1 = psum_mm()
            for k in range(2):
                nc.tensor.matmul(
                    p1[:1, :D], ecol[:, k, :], w1[:, k, :],
                    start=(k == 0), stop=(k == 1),
                )
            b1 = rowp.tile([1, D], F32, tag="b1row", bufs=1, name=f"b1_{tag}")
            nc.sync.dma_start(out=b1, in_=b1_h.ap())
            s1 = rowp.tile([1, D], F32, tag="s1row", bufs=1, name=f"s1_{tag}")
            nc.vector.tensor_tensor(out=s1, in0=p1[:1, :D], in1=b1, op=OP.add)
            s1b = rowp.tile([1, D], BF16, tag="s1brow", bufs=1, name=f"s1b_{tag}")
            nc.scalar.activation(s1b, s1, AF.Silu)
            s1c = rowp.tile([128, KT], BF16, tag="s1col", name=f"s1c_{tag}")
            row_to_col(s1b, s1c, D, BF16, f"s1_{tag}")
            w2 = const.tile([128, KT, D], BF16, tag="tw2", name=f"w2_{tag}")
            nc.sync.dma_start(out=w2, in_=w2_h.ap())
            p2 = psum_mm()
            for k in range(KT):
                nc.tensor.matmul(
                    p2[:1, :D], s1c[:, k : k + 1], w2[:, k, :],
                    start=(k == 0), stop=(k == KT - 1),
                )
            return p2

        p_t = t_embed(H["t_in"], H["t1w1_in"], H["t1w2_in"], H["t1b1_in"], "t")
        r_t = rowp.tile([1, D], F32, tag="rt", bufs=1)
        nc.scalar.copy(r_t, p_t[:1, :D])
        p_dt = t_embed(H["dt_in"], H["t2w1_in"], H["t2w2_in"], H["t2b1_in"], "dt")
        c_row = rowp.tile([1, D], F32, tag="crow", bufs=1)
        nc.vector.tensor_tensor(out=c_row, in0=r_t, in1=p_dt[:1, :D], op=OP.add)
        nc.vector.tensor_tensor(out=c_row, in0=c_row, in1=cbias_sb, op=OP.add)
        ca_row = rowp.tile([1, D], BF16, tag="carow", bufs=1)
        nc.scalar.activation(ca_row, c_row, AF.Silu)
        ca_col = persist.tile([128, KT], BF16)
        row_to_col(ca_row, ca_col, D, BF16, "ca")

        # =========================================================
        # Patch embed: hT = patch_w.T @ xp.T (+patch_b +posT)
        # =========================================================
        xph = const.tile([16, N], BF16)
        xpl = const.tile([16, N], BF16)
        pwh = const.tile([16, D], BF16)
        pwl = const.tile([16, D], BF16)
        patch_b = const.tile([128, KT], F32)
        nc.sync.dma_start(out=xph, in_=H["xpT_hi"].ap())
        nc.sync.dma_start(out=xpl, in_=H["xpT_lo"].ap())
        nc.sync.dma_start(out=pwh, in_=H["pw_hi"].ap())
        nc.sync.dma_start(out=pwl, in_=H["pw_lo"].ap())
        nc.sync.dma_start(out=patch_b, in_=H["patch_b_in"].ap())

        for mt in range(KT):
            pp = psum_s()
            for half in range(NHALF):
                sl = slice(half * 512, half * 512 + 512)
                for i, (w, x) in enumerate(((pwh, xph), (pwh, xpl), (pwl, xph))):
                    nc.tensor.matmul(
                        pp[:, sl],
                        w[:, mt * 128 : mt * 128 + 128],
                        x[:, sl],
                        start=(i == 0),
                        stop=(i == 2),
                    )
            post = resid.tile([128, N], F32, tag="res", name="post")
            nc.sync.dma_start(out=post, in_=H["posT_in"].ap()[:, mt, :])
            tmp = resid.tile([128, N], F32, tag="res", name="tmp")
            nc.vector.tensor_scalar_add(
                out=tmp, in0=pp, scalar1=patch_b[:, mt : mt + 1]
            )
            nc.vector.tensor_tensor(
                out=hT[:, mt, :], in0=tmp, in1=post, op=OP.add
            )

        if H.get("DBG"):
            dbg_h0_sb = resid.tile([128, KT, N], F32, tag="dbgbig", bufs=1, name="dbg_h0_sb")
            for mt in range(KT):
                nc.vector.tensor_copy(dbg_h0_sb[:, mt, :], hT[:, mt, :])
            nc.sync.dma_start(out=H["dbg_h0"].ap(), in_=dbg_h0_sb)

        # ---- adaLN modulation vectors (row-form matvec per layer) ----
        def emit_mod(l):
            mcol = stat.tile([128, 18], F32, tag="mcol", name=f"mcol{l}")
            nchunks = (6 * D + 511) // 512
            for ci in range(nchunks):
                cw = min(512, 6 * D - ci * 512)
                aw = wpool.tile(
                    [128, KT, 512], BF16, tag="adaln", name=f"aw{l}_{ci}"
                )
                nc.sync.dma_start(
                    out=aw[:, :, :cw],
                    in_=H["adaln_w_in"].ap()[l][:, :, ci * 512 : ci * 512 + cw],
                )
                pm = psum_mm("s")
                for k in range(KT):
                    nc.tensor.matmul(
                        pm[:1, :cw],
                        ca_col[:, k : k + 1],
                        aw[:, k, :cw],
                        start=(k == 0),
                        stop=(k == KT - 1),
                    )
                crow = rowp.tile([1, 512], F32, tag="rowbuf", name=f"mr{l}_{ci}")
                nc.scalar.copy(crow[:, :cw], pm[:1, :cw])
                row_to_col(
                    crow, mcol[:, ci * 4 : ci * 4 + cw // 128], cw, F32,
                    f"m{l}_{ci}",
                )
            ab = stat.tile([128, 18], F32, tag="abcol", name=f"ab{l}")
            nc.sync.dma_start(out=ab, in_=H["adaln_b_in"].ap()[l])
            nc.vector.tensor_tensor(
                out=mod_cols[:, l, :], in0=mcol, in1=ab, op=OP.add
            )
            # 1 + sc_a (cols 3:6), 1 + sc_m (cols 12:15)
            nc.vector.tensor_scalar_add(
                out=sc1_cols[:, l, 0:3], in0=mod_cols[:, l, 3:6], scalar1=1.0
            )
            nc.vector.tensor_scalar_add(
                out=sc1_cols[:, l, 3:6], in0=mod_cols[:, l, 12:15], scalar1=1.0
            )

        def emit_fin_mod():
            fcol_t = stat.tile([128, 6], F32, tag="mcol", name="fcol_t")
            for ci in range(2):
                cw = min(512, 2 * D - ci * 512)
                fw = wpool.tile(
                    [128, KT, 512], BF16, tag="adaln", name=f"fw{ci}"
                )
                nc.sync.dma_start(
                    out=fw[:, :, :cw],
                    in_=H["finmw_in"].ap()[:, :, ci * 512 : ci * 512 + cw],
                )
                pm = psum_mm("s")
                for k in range(KT):
                    nc.tensor.matmul(
                        pm[:1, :cw],
                        ca_col[:, k : k + 1],
                        fw[:, k, :cw],
                        start=(k == 0),
                        stop=(k == KT - 1),
                    )
                crow = rowp.tile([1, 512], F32, tag="rowbuf", name=f"fr{ci}")
                nc.scalar.copy(crow[:, :cw], pm[:1, :cw])
                row_to_col(
                    crow, fcol_t[:, ci * 4 : ci * 4 + cw // 128], cw, F32,
                    f"f{ci}",
                )
            fb = stat.tile([128, 6], F32, tag="abcol", name="fb")
            nc.sync.dma_start(out=fb, in_=H["finmb_in"].ap())
            nc.vector.tensor_tensor(out=fin_col, in0=fcol_t, in1=fb, op=OP.add)
            nc.vector.tensor_scalar_add(
                out=fin_sc1, in0=fin_col[:, 3:6], scalar1=1.0
            )

        emit_mod(0)
        if n_layers > 1:
            emit_mod(1)

        # =========================================================
        # LayerNorm -> modulated xm (T-domain)
        # =========================================================
        def layernorm_mod(xmTb, gamma, beta, xmLo=None, interleave=None):
            """xmTb[:, k, :] = LN_token(hT)*gamma + beta (bf16 out).

            meanB/sqB are produced directly as [128, 512] broadcast sums via
            rank-1 ones matmuls accumulated over the 3 k-tiles.
            """
            hTb = act16.tile([128, KT, N], BF16, tag="a16", bufs=3, name="hTb")
            sq = act16.tile([128, KT, N], BF16, tag="a16", bufs=3, name="sq")
            for k in range(KT):
                nc.vector.tensor_copy(hTb[:, k, :], hT[:, k, :])
                nc.vector.tensor_tensor(
                    out=sq[:, k, :], in0=hTb[:, k, :], in1=hTb[:, k, :],
                    op=OP.mult,
                )
            pM = [None, None]
            pQ = [None, None]
            for half in range(NHALF):
                sl = slice(half * 512, half * 512 + 512)
                pM[half] = psum_s([128, 512])
                for k in range(KT):
                    nc.tensor.matmul(
                        pM[half], ones_full, hTb[:, k, sl],
                        start=(k == 0), stop=(k == KT - 1),
                    )
                pQ[half] = psum_s([128, 512])
                for k in range(KT):
                    nc.tensor.matmul(
                        pQ[half], ones_full, sq[:, k, sl],
                        start=(k == 0), stop=(k == KT - 1),
                    )
                if half == 0 and interleave is not None:
                    interleave()
                msq = stat.tile([128, 512], F32, tag="statA", bufs=2, name="msq")
                nc.scalar.activation(msq, pM[half], AF.Square, scale=1.0 / D)
                var = stat.tile([128, 512], F32, tag="statA", bufs=2, name="var")
                nc.vector.scalar_tensor_tensor(
                    out=var, in0=pQ[half], scalar=1.0 / D, in1=msq,
                    op0=OP.mult, op1=OP.subtract,
                )
                sd = stat.tile([128, 512], F32, tag="statA", bufs=2, name="sd")
                nc.scalar.activation(sd, var, AF.Sqrt, bias=eps_col)
                rstd = stat.tile([128, 512], F32, tag="rstd", name="rstd")
                rscr = stat.tile([128, 512], F32, tag="statB", bufs=2, name="rscr")
                with nc.allow_low_precision(reason="rstd at ~2 ULP via NR"):
                    nc.vector.reciprocal_approx_accurate(
                        out=rstd, in_=sd, scratch=rscr
                    )
                for k in range(KT):
                    d_ = stat.tile([128, 512], F32, tag="statB", bufs=2, name="d_")
                    nc.vector.scalar_tensor_tensor(
                        out=d_, in0=pM[half], scalar=-1.0 / D, in1=hT[:, k, sl],
                        op0=OP.mult, op1=OP.add,
                    )
                    e_ = stat.tile([128, 512], F32, tag="statB", bufs=2, name="e_")
                    nc.vector.tensor_tensor(out=e_, in0=d_, in1=rstd, op=OP.mult)
                    if xmLo is None:
                        nc.vector.tensor_scalar(
                            out=xmTb[:, k, sl], in0=e_,
                            scalar1=gamma[:, k : k + 1], scalar2=beta[:, k : k + 1],
                            op0=OP.mult, op1=OP.add,
                        )
                    else:
                        xf = stat.tile(
                            [128, 512], F32, tag="statB", bufs=2, name="xf"
                        )
                        nc.vector.tensor_scalar(
                            out=xf, in0=e_,
                            scalar1=gamma[:, k : k + 1], scalar2=beta[:, k : k + 1],
                            op0=OP.mult, op1=OP.add,
                        )
                        nc.vector.tensor_copy(xmTb[:, k, sl], xf)
                        nc.vector.scalar_tensor_tensor(
                            out=xmLo[:, k, sl], in0=xmTb[:, k, sl], scalar=-1.0,
                            in1=xf, op0=OP.mult, op1=OP.add,
                        )

        # =========================================================
        # Transformer layers
        # =========================================================
        for l in range(n_layers):
            mc = mod_cols[:, l, :]
            sc1 = sc1_cols[:, l, :]
            # ---- LN1 + modulation ----
            xmTb = act16.tile([128, KT, N], BF16, tag="a16", bufs=3, name="xmTb")
            layernorm_mod(xmTb, sc1[:, 0:3], mc[:, 0:3])

            # ---- qkv ----
            qw = wpool.tile([128, KT, D], BF16, tag="qw", name=f"qw{l}")
            kw = wpool.tile([128, KT, D], BF16, tag="kw", name=f"kw{l}")
            vw = wpool.tile([128, KT, D], BF16, tag="vw", name=f"vw{l}")
            nc.sync.dma_start(out=qw, in_=H["qw_in"].ap()[l])
            nc.sync.dma_start(out=kw, in_=H["kw_in"].ap()[l])
            nc.sync.dma_start(out=vw, in_=H["vw_in"].ap()[l])
            qb = stat.tile([128, KT], F32, tag="bcol", bufs=4, name=f"qb{l}")
            kb = stat.tile([128, KT], F32, tag="bcol", bufs=4, name=f"kb{l}")
            nc.sync.dma_start(out=qb, in_=H["qb_in"].ap()[l])
            nc.sync.dma_start(out=kb, in_=H["kb_in"].ap()[l])
            vbB = stat.tile([128, D], F32, tag="vbB", bufs=1, name=f"vbB{l}")
            nc.sync.dma_start(
                out=vbB,
                in_=bass.AP(
                    tensor=H["vb_row_in"].ap().tensor,
                    offset=l * D,
                    ap=[[0, 128], [1, D]],
                ),
            )

            for mt in range(KT):
                pq = psum_s()
                for half in range(NHALF):
                    sl = slice(half * 512, half * 512 + 512)
                    for k in range(KT):
                        nc.tensor.matmul(
                            pq[:, sl], qw[:, k, mt * 128 : mt * 128 + 128],
                            xmTb[:, k, sl],
                            start=(k == 0), stop=(k == KT - 1),
                        )
                nc.vector.tensor_scalar(
                    out=qTb[:, mt, :], in0=pq,
                    scalar1=qb[:, mt : mt + 1], scalar2=1.0 / HD,
                    op0=OP.add, op1=OP.mult,
                )
                pk = psum_s()
                for half in range(NHALF):
                    sl = slice(half * 512, half * 512 + 512)
                    for k in range(KT):
                        nc.tensor.matmul(
                            pk[:, sl], kw[:, k, mt * 128 : mt * 128 + 128],
                            xmTb[:, k, sl],
                            start=(k == 0), stop=(k == KT - 1),
                        )
                nc.vector.tensor_scalar_add(
                    out=kTb[:, mt, :], in0=pk, scalar1=kb[:, mt : mt + 1]
                )
            # v in normal layout [token, d] with bias, strided into v_ext
            for jt in range(8):
                pv = psum_mm()
                jsl = slice(jt * 128, jt * 128 + 128)
                for k in range(KT):
                    nc.tensor.matmul(
                        pv[:, :D], xmTb[:, k, jsl], vw[:, k, :],
                        start=(k == 0), stop=(k == KT - 1),
                    )
                vdst = bass.AP(
                    tensor=v_ext.tensor,
                    offset=v_ext.offset + jt * (6 * 65),
                    ap=[list(v_ext.ap[0]), [65, 6], [1, 64]],
                )
                pv_v = bass.AP(
                    tensor=pv.tensor, offset=pv.offset,
                    ap=[list(pv.ap[0]), [64, 6], [1, 64]],
                )
                vb_v = bass.AP(
                    tensor=vbB.tensor, offset=vbB.offset,
                    ap=[list(vbB.ap[0]), [64, 6], [1, 64]],
                )
                nc.vector.tensor_tensor(out=vdst, in0=pv_v, in1=vb_v, op=OP.add)

            # ---- attention: process heads in pairs (PE row-tiling:
            # even head occupies rows 0-63, odd head rows 64-127, their
            # score matmuls run concurrently on the array) ----
            for hp in range(NH // 2):
                mt = hp  # head pair hp = heads (2hp, 2hp+1); mt = hp
                po_oA = psum_o()
                po_oB = psum_o()
                for jt in range(8):
                    jsl = slice(jt * 128, jt * 128 + 128)
                    ps_sA = psum_s()
                    ps_sB = psum_s()
                    for half in range(NHALF):
                        sl = slice(half * 512, half * 512 + 512)
                        nc.tensor.matmul(
                            ps_sA[:, sl],
                            kTb[0:64, mt, jsl],
                            qTb[0:64, mt, sl],
                            start=True, stop=True,
                        )
                        nc.tensor.matmul(
                            ps_sB[:, sl],
                            kTb[64:128, mt, jsl],
                            qTb[64:128, mt, sl],
                            start=True, stop=True,
                        )
                    expA = expp.tile([128, N], BF16, tag="expT", name="expA")
                    nc.scalar.activation(expA, ps_sA, AF.Exp)
                    expB = expp.tile([128, N], BF16, tag="expT", name="expB")
                    nc.scalar.activation(expB, ps_sB, AF.Exp)
                    for half in range(NHALF):
                        sl = slice(half * 512, half * 512 + 512)
                        nc.tensor.matmul(
                            po_oA[:, sl],
                            v_ext[:, jt, (2 * hp) * 65 : (2 * hp) * 65 + 65],
                            expA[:, sl],
                            start=(jt == 0), stop=(jt == 7),
                        )
                        nc.tensor.matmul(
                            po_oB[:, sl],
                            v_ext[:, jt, (2 * hp + 1) * 65 : (2 * hp + 1) * 65 + 65],
                            expB[:, sl],
                            start=(jt == 0), stop=(jt == 7),
                        )
                # Batched 1/Z for the pair: rows 0-63 broadcast Z_A, rows
                # 64-127 broadcast Z_B; one reciprocal covers both heads.
                zrowA = rowp.tile([1, N], F32, tag="zrow", bufs=1, name="zrowA")
                nc.scalar.copy(zrowA, po_oA[64:65, :])
                zrowB = rowp.tile([1, N], F32, tag="zrowB", bufs=1, name="zrowB")
                nc.vector.tensor_copy(zrowB, po_oB[64:65, :])
                zscrA = dramp.tile([N], F32, tag="zscr", name="zscrA")
                nc.sync.dma_start(out=zscrA, in_=zrowA[:1, :])
                zscrB = dramp.tile([N], F32, tag="zscrB", name="zscrB")
                nc.sync.dma_start(out=zscrB, in_=zrowB[:1, :])
                zbB = expp.tile([128, N], F32, tag="zbB", bufs=1, name="zbB")
                nc.sync.dma_start(
                    out=zbB[0:64, :],
                    in_=bass.AP(
                        tensor=zscrA.tensor, offset=zscrA.offset,
                        ap=[[0, 64], [1, N]],
                    ),
                )
                nc.sync.dma_start(
                    out=zbB[64:128, :],
                    in_=bass.AP(
                        tensor=zscrB.tensor, offset=zscrB.offset,
                        ap=[[0, 64], [1, N]],
                    ),
                )
                zrec = expp.tile([128, N], F32, tag="zrec", bufs=1, name="zrec")
                with nc.allow_low_precision(reason="1/Z at 18 bits"):
                    nc.vector.reciprocal_approx_fast(out=zrec, in_=zbB)
                nc.vector.tensor_tensor(
                    out=oTb[0:64, mt, :],
                    in0=po_oA[0:64, :], in1=zrec[0:64, :], op=OP.mult,
                )
                nc.vector.tensor_tensor(
                    out=oTb[64:128, mt, :],
                    in0=po_oB[0:64, :], in1=zrec[64:128, :], op=OP.mult,
                )

            # adaLN matvecs for layer l+2 fill the attention-tail PE bubble
            if l + 2 < n_layers:
                emit_mod(l + 2)
            elif l + 2 == n_layers:
                emit_fin_mod()

            # ---- o-proj + residual ----
            # k=0,1 (head pairs 0/1, already finished) are emitted first so
            # they run while the last pair's 1/Z chain completes; the k=2
            # matmuls join as pair 2 lands.
            ow = wpool.tile([128, KT, D], BF16, tag="ow", name=f"ow{l}")
            nc.sync.dma_start(out=ow, in_=H["ow_in"].ap()[l])
            ob = stat.tile([128, KT], F32, tag="bcol", bufs=4, name=f"ob{l}")
            nc.sync.dma_start(out=ob, in_=H["ob_in"].ap()[l])

            def oproj_mm(pr, mt, k, first, last):
                for half in range(NHALF):
                    sl = slice(half * 512, half * 512 + 512)
                    nc.tensor.matmul(
                        pr[:, sl], ow[:, k, mt * 128 : mt * 128 + 128],
                        oTb[:, k, sl], start=first, stop=last,
                    )

            def oproj_evac(pr, mt):
                tmp = resid.tile([128, N], F32, tag="res", name="tmpo")
                nc.vector.tensor_scalar(
                    out=tmp, in0=pr,
                    scalar1=ob[:, mt : mt + 1], scalar2=mc[:, 6 + mt : 7 + mt],
                    op0=OP.add, op1=OP.mult,
                )
                nc.vector.tensor_tensor(
                    out=hT[:, mt, :], in0=hT[:, mt, :], in1=tmp, op=OP.add
                )

            pr0 = psum_s()
            for k in range(2):
                oproj_mm(pr0, 0, k, k == 0, False)
            pr1 = psum_s()
            for k in range(2):
                oproj_mm(pr1, 1, k, k == 0, False)
            oproj_mm(pr0, 0, 2, False, True)
            oproj_evac(pr0, 0)
            pr2 = psum_s()
            for k in range(KT):
                oproj_mm(pr2, 2, k, k == 0, k == KT - 1)
            oproj_mm(pr1, 1, 2, False, True)
            oproj_evac(pr1, 1)
            oproj_evac(pr2, 2)

            # ---- LN2 + MLP ----
            xm2Tb = act16.tile([128, KT, N], BF16, tag="a16", bufs=3, name="xm2Tb")
            layernorm_mod(xm2Tb, sc1[:, 3:6], mc[:, 9:12])

            mw1 = wpool.tile([128, KT, MLP], BF16, tag="mw1", name=f"mw1{l}")
            mw2 = wpool.tile([128, MT1, D], BF16, tag="mw2", name=f"mw2{l}")
            nc.sync.dma_start(out=mw1, in_=H["mw1_in"].ap()[l])
            nc.sync.dma_start(out=mw2, in_=H["mw2_in"].ap()[l])
            mb1 = stat.tile([128, MT1], F32, tag="mb1col", name=f"mb1{l}")
            mb2 = stat.tile([128, KT], F32, tag="bcol", bufs=4, name=f"mb2{l}")
            nc.sync.dma_start(out=mb1, in_=H["mb1_in"].ap()[l])
            nc.sync.dma_start(out=mb2, in_=H["mb2_in"].ap()[l])

            for half in range(NHALF):
                sl = slice(half * 512, half * 512 + 512)
                gTb = act16.tile(
                    [128, MT1, 512], BF16, tag="gTb", name="gTb"
                )
                for mt in range(MT1):
                    pg = psum_mm()
                    for k in range(KT):
                        nc.tensor.matmul(
                            pg, mw1[:, k, mt * 128 : mt * 128 + 128],
                            xm2Tb[:, k, sl],
                            start=(k == 0), stop=(k == KT - 1),
                        )
                    nc.scalar.activation(
                        gTb[:, mt, :], pg, AF.Gelu, bias=mb1[:, mt : mt + 1]
                    )
                for mt in range(KT):
                    pf = psum_mm()
                    for k in range(MT1):
                        nc.tensor.matmul(
                            pf, mw2[:, k, mt * 128 : mt * 128 + 128],
                            gTb[:, k, :],
                            start=(k == 0), stop=(k == MT1 - 1),
                        )
                    tmp2 = resid.tile([128, 512], F32, tag="resh", name="tmpm")
                    nc.vector.tensor_scalar(
                        out=tmp2, in0=pf,
                        scalar1=mb2[:, mt : mt + 1],
                        scalar2=mc[:, 15 + mt : 16 + mt],
                        op0=OP.add, op1=OP.mult,
                    )
                    nc.vector.tensor_tensor(
                        out=hT[:, mt, sl], in0=hT[:, mt, sl], in1=tmp2, op=OP.add
                    )

        # =========================================================
        # Final layer
        # =========================================================
        if n_layers < 2:
            emit_fin_mod()
        xmF = act16.tile([128, KT, N], BF16, tag="a16", bufs=3, name="xmF")
        xmFlo = act16.tile([128, KT, N], BF16, tag="a16", bufs=3, name="xmFlo")
        layernorm_mod(xmF, fin_sc1, fin_col[:, 0:3], xmLo=xmFlo)
        if H.get("DBG"):
            nc.sync.dma_start(out=H["dbg_fincol"].ap(), in_=fin_col)
            dbg_xm_sb = resid.tile([128, KT, N], F32, tag="dbgbig", bufs=1, name="dbg_xm_sb")
            for mt in range(KT):
                nc.vector.tensor_copy(dbg_xm_sb[:, mt, :], xmF[:, mt, :])
            nc.sync.dma_start(out=H["dbg_xmF"].ap(), in_=dbg_xm_sb)
            dbg_ca_sb = resid.tile([128, KT], F32, tag="dbgca", bufs=1, name="dbg_ca_sb")
            nc.vector.tensor_copy(dbg_ca_sb, ca_col)
            nc.sync.dma_start(out=H["dbg_ca"].ap(), in_=dbg_ca_sb)
        # hi/lo split of xmF for the precision-critical final matmul:
        # xmF is already bf16; recompute hi=xmF (bf16) and lo = fp32(xm) - hi.
        # We recompute the fp32 xm into a temp to form lo.
        # Cheaper: lo = (e2*gamma+beta) - xmF computed per tile below.
        fpwh = const.tile([128, KT, 16], BF16)
        fpwl = const.tile([128, KT, 16], BF16)
        fpb = const.tile([16, 1], F32)
        nc.sync.dma_start(out=fpwh, in_=H["fpw_hi"].ap())
        nc.sync.dma_start(out=fpwl, in_=H["fpw_lo"].ap())
        nc.sync.dma_start(out=fpb, in_=H["fpb_in"].ap())

        pout = psum_mm()
        pout2 = psum_mm()
        pps = [pout, pout2]
        for half in range(NHALF):
            sl = slice(half * 512, half * 512 + 512)
            mms = []
            for k in range(KT):
                mms.append((fpwh[:, k, :], xmF[:, k, sl]))
                mms.append((fpwl[:, k, :], xmF[:, k, sl]))
                mms.append((fpwh[:, k, :], xmFlo[:, k, sl]))
            for i, (wv, xv) in enumerate(mms):
                nc.tensor.matmul(
                    pps[half][:16, :], wv, xv,
                    start=(i == 0), stop=(i == len(mms) - 1),
                )
        out_sb = resid.tile([16, N], F32, tag="outsb", bufs=1, name="out_sb")
        for half in range(NHALF):
            sl = slice(half * 512, half * 512 + 512)
            nc.vector.tensor_scalar_add(
                out=out_sb[:, sl], in0=pps[half][:16, :], scalar1=fpb
            )
        nc.sync.dma_start(out=H["outT"].ap(), in_=out_sb)


# =================================================================
# Host side
# =================================================================
_BUILD_CACHE = {}


def _get_module(n_layers=L):
    if n_layers not in _BUILD_CACHE:
        _register_profile_hook()
        _BUILD_CACHE[n_layers] = build_module(n_layers)
    return _BUILD_CACHE[n_layers]


def _shuf_w(w):
    """[Din, Dout] -> [128, Din//128, Dout], partition-contiguous."""
    din = w.shape[0]
    return np.ascontiguousarray(
        w.reshape(din // 128, 128, -1).transpose(1, 0, 2)
    )


def _col(v):
    """[D] -> [128, D//128] column layout."""
    return np.ascontiguousarray(v.reshape(-1, 128).T)


def _bf(x):
    return np.asarray(x, np.float32).astype(ml_dtypes.bfloat16)


def prepare_inputs(inputs, n_layers=L):
    """Build the 8 per-core in_maps from the full input dict."""
    ii = {k: np.asarray(v) for k, v in inputs.items()}
    x = ii["x"].astype(np.float32)
    t = ii["t"].astype(np.float32)
    dt = ii["dt"].astype(np.float32)
    y = ii["y"].astype(np.int64)
    pos = ii["pos"].astype(np.float32)

    shared = {}
    shared["posT"] = _shuf_w(np.ascontiguousarray(pos.T))
    pw = ii["patch_w"].astype(np.float32)
    pwh = pw.astype(ml_dtypes.bfloat16)
    shared["pw_hi"] = pwh
    shared["pw_lo"] = (pw - pwh.astype(np.float32)).astype(ml_dtypes.bfloat16)
    shared["patch_b"] = _col(ii["patch_b"].astype(np.float32))
    shared["t1w1"] = _shuf_w(_bf(ii["t1_w1"]))
    shared["t1w2"] = _shuf_w(_bf(ii["t1_w2"]))
    shared["t2w1"] = _shuf_w(_bf(ii["t2_w1"]))
    shared["t2w2"] = _shuf_w(_bf(ii["t2_w2"]))
    shared["t1b1"] = ii["t1_b1"].astype(np.float32).reshape(1, D)
    shared["t2b1"] = ii["t2_b1"].astype(np.float32).reshape(1, D)
    half = FREQ // 2
    shared["freqs"] = np.exp(
        -math.log(10000.0) * np.arange(half, dtype=np.float64) / half
    ).astype(np.float32).reshape(half, 1)
    shared["adaln_w"] = np.stack([_shuf_w(_bf(ii["adaln_w"][l])) for l in range(L)])
    shared["adaln_b"] = np.stack(
        [_col(ii["adaln_b"][l].astype(np.float32)) for l in range(L)]
    )
    for nm, src in (("qw", "q_w"), ("kw", "k_w"), ("vw", "v_w"), ("ow", "o_w")):
        shared[nm] = np.stack([_shuf_w(_bf(ii[src][l])) for l in range(L)])
    for nm, src in (("qb", "q_b"), ("kb", "k_b"), ("ob", "o_b"), ("mb2", "m_b2")):
        shared[nm] = np.stack(
            [_col(ii[src][l].astype(np.float32)) for l in range(L)]
        )
    shared["vb_row"] = ii["v_b"].astype(np.float32).reshape(L, 1, D)
    shared["mw1"] = np.stack([_shuf_w(_bf(ii["m_w1"][l])) for l in range(L)])
    shared["mw2"] = np.stack([_shuf_w(_bf(ii["m_w2"][l])) for l in range(L)])
    shared["mb1"] = np.stack(
        [_col(ii["m_b1"][l].astype(np.float32)) for l in range(L)]
    )
    shared["finmw"] = _shuf_w(_bf(ii["fin_mw"]))
    shared["finmb"] = _col(ii["fin_mb"].astype(np.float32))
    fpw = _shuf_w(ii["fin_pw"].astype(np.float32))
    fpwh = fpw.astype(ml_dtypes.bfloat16)
    shared["fpw_hi"] = fpwh
    shared["fpw_lo"] = (fpw - fpwh.astype(np.float32)).astype(ml_dtypes.bfloat16)
    shared["fpb"] = ii["fin_pb"].astype(np.float32).reshape(16, 1)

    label_emb = ii["label_emb"].astype(np.float32)
    cb_common = (ii["t1_b2"].astype(np.float32) + ii["t2_b2"].astype(np.float32))

    in_maps = []
    for b in range(B):
        m = dict(shared)
        xp = (
            x[b]
            .reshape(CIN, HP, P, HP, P)
            .transpose(1, 3, 0, 2, 4)
            .reshape(N, CIN * P * P)
        )
        xpT = np.ascontiguousarray(xp.T)
        xph = xpT.astype(ml_dtypes.bfloat16)
        m["xpT_hi"] = xph
        m["xpT_lo"] = (xpT - xph.astype(np.float32)).astype(ml_dtypes.bfloat16)
        m["t_in"] = t[b].reshape(1, 1)
        m["dt_in"] = dt[b].reshape(1, 1)
        m["cbias"] = (cb_common + label_emb[int(y[b])]).reshape(1, D)
        in_maps.append(m)
    return in_maps


def assemble_output(results):
    out = np.empty((B, COUT, IMG, IMG), np.float32)
    for b in range(B):
        tok = results[b]["outT"].T  # [N, 16]
        out[b] = (
            tok.reshape(HP, HP, P, P, COUT)
            .transpose(4, 0, 2, 1, 3)
            .reshape(COUT, IMG, IMG)
        )
    return out


def run(inputs, n_layers=L, trace=False, sim=False):
    nc = _get_module(n_layers)
    in_maps = prepare_inputs(inputs, n_layers)
    if sim:
        from concourse.bass_interp import CoreSim

        s = CoreSim(nc, trace=False)
        for k, v in in_maps[0].items():
            s.tensor(k)[:] = v
        s.simulate()
        names = ["outT"]
        if os.environ.get("DIT_DBG", "0") == "1":
            names += ["dbg_h0", "dbg_fincol", "dbg_xmF", "dbg_ca"]
        results = [
            {n: np.array(s.tensor(n)) for n in names} for _ in range(B)
        ]
        return results, None
    res = run_bass_kernel_spmd(
        nc, in_maps, core_ids=list(range(B)), trace=trace
    )
    return res.results, res


def kernel(**inputs):
    results, _ = run(inputs, L, trace=False, sim=False)
    return assemble_output(results)
